# revision 11
# baseline (speedup 1.0000x reference)
"""Trainium2 Bass kernel for nn_MultiHeadMuonLoRALinear.

Math: out = x @ W^T + bias + sum_h alpha_h * x @ M_h^T, where
M_h = newtonschulz5(B_h @ A_h) and G_h = B_h @ A_h has rank hr=4.

Key algebraic identity: with G = B A (rank hr), every Newton-Schulz
iterate stays in the same row/column space, so X_k = B C_k A for an
hr x hr matrix C_k:
    C_0 = I / (||G||_F + eps),  ||G||_F^2 = tr((B^T B)(A A^T))
    C'  = a C + b (C P C^T) Q C + c (C P C^T Q)^2 C,  P = A A^T, Q = B^T B
Therefore M_h = B_h C_h A_h and the whole LoRA branch collapses to a
rank-16 update:  delta = sum_h alpha_h B_h C_h A_h,  out = x @ (W + delta)^T + bias.

The device kernel computes the single large GEMM (data-parallel over
tokens across 8 cores) with the rank-16 delta folded into W on the
host (0.2% of total FLOPs) and the bias fused into the PSUM->SBUF
copy on the scalar engine. Matmuls run in float32r (fp32 data path,
fp22 multiply) at full PE rate.
"""

import numpy as np

import concourse.bass as bass
import concourse.bacc as bacc
import concourse.mybir as mybir
import concourse.tile as tile
from concourse.bass import ts
from concourse.bass_utils import run_bass_kernel_spmd

N_HEADS = 4
NS_STEPS = 5
NS_EPS = 1e-7
NS_A, NS_B, NS_C = 3.4445, -4.775, 2.0315

N_CORES = 8
P = 128

F32 = mybir.dt.float32
F32R = mybir.dt.float32r


def host_fold_lora(W, bias, lora_A, lora_B):
    """Collapse the per-head Newton-Schulz into hr x hr space (float64)
    and return W_eff = W + sum_h alpha_h B_h C_h A_h (float32)."""
    r, D_in = lora_A.shape
    D_out = lora_B.shape[0]
    hr = r // N_HEADS
    Ah = lora_A.reshape(N_HEADS, hr, D_in).astype(np.float64)
    Bh = lora_B.reshape(D_out, N_HEADS, hr).transpose(1, 0, 2).astype(np.float64)

    AT = np.zeros((r, D_in))   # rows: alpha-weighted C_h A_h per head
    BT = np.zeros((D_out, r))  # cols: B_h per head
    for h in range(N_HEADS):
        A = Ah[h]
        B = Bh[h]
        Pm = A @ A.T
        Qm = B.T @ B
        fro = np.sqrt(np.trace(Qm @ Pm))
        C = np.eye(hr) / (fro + NS_EPS)
        for _ in range(NS_STEPS):
            D = C @ Pm @ C.T
            E = D @ Qm
            C = NS_A * C + NS_B * (E @ C) + NS_C * (E @ (E @ C))
        AT[h * hr:(h + 1) * hr] = fro * (C @ A)
        BT[:, h * hr:(h + 1) * hr] = B
    delta = BT @ AT
    return (W.astype(np.float64) + delta).astype(np.float32)


def build_bass(K, O, T, t_block, phase_a_ot=3, w_dtype=None, warmup=40):
    """Per-core SPMD program: outT[O, T] = (x W_eff^T + bias)^T for this
    core's token shard.

    DRAM layouts (host-prepared, partition-major contiguous):
      x:    [K//128, 128, T]      x_dev[kt, k, t] = x_shard[t, kt*128 + k]
      w:    [O//128, 128, K//128, 128]  w_dev[ot, k, kt, o] = W_eff[ot*128+o, kt*128+k]
      bias: [128, O//128]         bias_dev[o, ot] = bias[ot*128 + o]
      out:  [O, T]                outT

    x streams on the Sync HWDGE ring while W slabs + bias go on the
    Scalar HWDGE ring, so the first weight slab doesn't queue behind
    16MB of x. Phase A k-sweeps the first `phase_a_ot` o-tiles across
    parallel PSUM banks so the PE consumes x tiles as they land; the
    remaining o-tiles run k-contiguous (PE stays warm, one PSUM group
    at a time).
    """
    KT, OT = K // P, O // P
    TB = T // t_block
    A = min(phase_a_ot, OT)
    if w_dtype is None:
        w_dtype = F32R
    nc = bacc.Bacc()

    x_d = nc.declare_dram_parameter("x", [KT, P, T], F32R, isOutput=False)
    w_d = nc.declare_dram_parameter("w", [OT, P, KT, P], w_dtype, isOutput=False)
    b_d = nc.declare_dram_parameter("bias", [P, OT], F32, isOutput=False)
    out_d = nc.declare_dram_parameter("out", [O, T], F32, isOutput=True)

    with tile.TileContext(nc) as tc:
        with (
            tc.tile_pool(name="xpool", bufs=1) as xpool,
            tc.tile_pool(name="cpool", bufs=1) as cpool,
            tc.tile_pool(name="wpool", bufs=max(A, 2)) as wpool,
            tc.tile_pool(name="opool", bufs=2) as opool,
            tc.tile_pool(name="pspool", bufs=8, space="PSUM") as pspool,
        ):
            bias_sb = cpool.tile([P, OT], F32)
            nc.scalar.dma_start(out=bias_sb[:], in_=b_d[:])

            def load_w(ot):
                wt = wpool.tile([P, KT, P], w_dtype, tag="w", name=f"w{ot}")
                nc.scalar.dma_start(out=wt[:], in_=w_d[ot])
                return wt

            w_a = [load_w(ot) for ot in range(A)]

            if warmup:
                # PE warmup: dependency-free matmuls on a memset tile keep
                # the PE busy through the HAM SHORT window while x/W stream
                # in, so phase A runs at 2.4 GHz from its first matmul.
                wu_src = cpool.tile([P, t_block], F32, name="wu_src")
                nc.vector.memset(wu_src[:], 0.0)
                wu_ps = pspool.tile([P, t_block], F32, tag="ps", name="wu_ps")
                wu_r = wu_src.bitcast(F32R)
                for _ in range(warmup):
                    nc.tensor.matmul(
                        wu_ps[:], lhsT=wu_r[:, :P], rhs=wu_r[:],
                        start=True, stop=True,
                    )

            x_tiles = []
            for kt in range(KT):
                xt = xpool.tile([P, T], F32R, tag=f"x{kt}", name=f"x{kt}")
                nc.sync.dma_start(out=xt[:], in_=x_d[kt])
                x_tiles.append(xt)

            def emit_out(ot, ps_list):
                out_sb = opool.tile([P, T], F32)
                for tb in range(TB):
                    nc.scalar.activation(
                        out_sb[:, ts(tb, t_block)],
                        ps_list[tb][:],
                        mybir.ActivationFunctionType.Identity,
                        bias=bias_sb[:, ot:ot + 1],
                    )
                nc.sync.dma_start(out=out_d[ts(ot, P), :], in_=out_sb[:])

            # Phase A: k-outer sweep over the first A o-tiles in parallel
            # PSUM banks, consuming each x tile as soon as it lands.
            ps_a = [[pspool.tile([P, t_block], F32, tag="ps",
                                 name=f"psA{ot}_{tb}")
                     for tb in range(TB)] for ot in range(A)]
            for kt in range(KT):
                for ot in range(A):
                    for tb in range(TB):
                        nc.tensor.matmul(
                            ps_a[ot][tb][:],
                            lhsT=w_a[ot][:, kt, :],
                            rhs=x_tiles[kt][:, ts(tb, t_block)],
                            start=(kt == 0),
                            stop=(kt == KT - 1),
                        )
            for ot in range(A):
                emit_out(ot, ps_a[ot])

            # Phase B: k-contiguous, one o-tile at a time.
            for ot in range(A, OT):
                wt = load_w(ot)
                ps_list = []
                for tb in range(TB):
                    ps = pspool.tile([P, t_block], F32, tag="ps", name=f"ps{ot}_{tb}")
                    for kt in range(KT):
                        nc.tensor.matmul(
                            ps[:],
                            lhsT=wt[:, kt, :],
                            rhs=x_tiles[kt][:, ts(tb, t_block)],
                            start=(kt == 0),
                            stop=(kt == KT - 1),
                        )
                    ps_list.append(ps)
                emit_out(ot, ps_list)

    nc.compile()
    return nc


def _prep_core_inputs(x2d, W_eff, bias, K, O, T, n_cores, w_np=np.float32):
    """Host-side layout prep: shard tokens, make partition-major layouts."""
    KT, OT = K // P, O // P
    w_dev = np.ascontiguousarray(
        W_eff.reshape(OT, P, KT, P).transpose(0, 3, 2, 1).astype(w_np)
    )  # [ot, k, kt, o]
    bias_dev = np.ascontiguousarray(bias.reshape(OT, P).T)  # [o(part), ot]
    in_maps = []
    for c in range(n_cores):
        xs = x2d[c * T:(c + 1) * T]  # [T, K]
        x_dev = np.ascontiguousarray(xs.reshape(T, KT, P).transpose(1, 2, 0))
        in_maps.append({"x": x_dev, "w": w_dev, "bias": bias_dev})
    return in_maps


W_FP16 = False  # fp16 stationary is rejected by walrus when mixed with f32r


def kernel(x, W, bias, lora_A, lora_B, trace=False, _nc_cache={}):
    x = np.asarray(x, np.float32)
    W = np.asarray(W, np.float32)
    bias = np.asarray(bias, np.float32)
    lora_A = np.asarray(lora_A, np.float32)
    lora_B = np.asarray(lora_B, np.float32)
    B, S, D_in = x.shape
    D_out = bias.shape[0]
    T_total = B * S
    T = T_total // N_CORES

    W_eff = host_fold_lora(W, bias, lora_A, lora_B)
    x2d = np.ascontiguousarray(x.reshape(T_total, D_in))

    w_mb, w_np = (mybir.dt.float16, np.float16) if W_FP16 else (F32R, np.float32)
    key = (D_in, D_out, T, w_mb)
    if key not in _nc_cache:
        _nc_cache[key] = build_bass(D_in, D_out, T, 512, phase_a_ot=4,
                                    w_dtype=w_mb)
    nc = _nc_cache[key]

    in_maps = _prep_core_inputs(x2d, W_eff, bias, D_in, D_out, T, N_CORES,
                                w_np=w_np)
    res = run_bass_kernel_spmd(nc, in_maps, list(range(N_CORES)), trace=trace)

    out = np.empty((T_total, D_out), dtype=np.float32)
    for c in range(N_CORES):
        out[c * T:(c + 1) * T] = res.results[c]["out"].T
    out = out.reshape(B, S, D_out)
    if trace:
        return out, res
    return out
